# revision 1
# baseline (speedup 1.0000x reference)
"""Multi-head linear attention on Trainium2 — 8-core SPMD, batch+head sharded.

Full-tensor contract: kernel(**inputs) takes the complete Q/K/V
[4, 4096, 1024] f32 arrays, internally shards them across 8 NeuronCores
(core c -> batch c//2, heads 8*(c%2) .. 8*(c%2)+8, i.e. a contiguous
512-column slice of the embedding dim), runs one Bass kernel per core,
and reassembles the full [4, 4096, 1024] f32 output.

Per-core math (H=8 local heads, D=64, L=4096):
    phi = sigmoid(0.6053*x - 4.102)
    kv_ext[h] = phi_K[h]^T @ [V[h] | 1]     # [64, 65], f32 PSUM accum
    numden[h] = phi_Q[h] @ kv_ext[h]        # [L, 65]
    out[h]    = numden[h][:, :64] / numden[h][:, 64:65]

Layout: the host restacks each core's [4096, 512] slice to [8192, 256]
(head groups 0-3 / 4-7 stacked along rows) so the two 4-head groups
pipeline — group 0's phase-Q and division overlap group 1's K/V
streaming — while every DMA still moves 2 KiB-contiguous lines (each
SBUF partition line carries 2 consecutive L-rows; kv accumulation over
L is order-invariant, and the q-row permutation is undone because the
output uses the same 2-rows-per-partition layout the host unstacks).

Heads are processed in pairs: one K=128 matmul per pair computes both
heads' kv_ext blocks (phi_K pair chunk stationary, [V|1] pair moving;
off-diagonal blocks discarded), and one K=128 matmul per pair computes
both numden blocks against a block-diagonal kv operand. Q is
transposed raw on the PE (f32), sigmoid fuses the PSUM->SBUF copy on
ScalarE, V is cast f32->bf16 in-flight by SWDGE DMA. The division runs
on VectorE: per-chunk PSUM->SBUF copy, one batched reciprocal, one
broadcast multiply per row-tile. Matmul inputs are bf16 (PSUM
accumulation stays f32).
"""

import numpy as np

B = 4
L = 4096
E = 1024
NH = 8            # heads per core
D = 64
W = D + 1         # head block width incl. ones/den column
EC = NH * D       # 512 embedding columns per core
P = 128
G = 2             # head groups (4 heads each), stacked along rows
GC = EC // G      # 256 columns per group
NPAIR = GC // P   # head pairs per group (2)
SUB = 2           # L-rows per partition line (512 f32 = 2 KiB)
RT = SUB * GC     # 512 elements per partition line
NT = L // (P * SUB)   # 16 super-tiles (256 L-rows) per group
TBS = 4           # super-tiles per DMA batch -> 1 MiB loads
NBS = NT // TBS   # 4 batches per tensor per group
N_CORES = 8

_CACHE = {}


def _build_nc():
    from contextlib import ExitStack

    import concourse.bacc as bacc
    import concourse.bass as bass
    import concourse.mybir as mybir
    import concourse.tile as tile
    from concourse.masks import make_identity

    f32 = mybir.dt.float32
    bf16 = mybir.dt.bfloat16
    SIG = mybir.ActivationFunctionType.Sigmoid

    nc = bacc.Bacc("TRN2", target_bir_lowering=False, debug=False)
    Q = nc.dram_tensor("Q", [EC, L], f32, kind="ExternalInput").ap()
    K = nc.dram_tensor("K", [G * L, GC], f32, kind="ExternalInput").ap()
    V = nc.dram_tensor("V", [G * L, GC], f32, kind="ExternalInput").ap()
    O = nc.dram_tensor("O", [2 * G * L, P], f32, kind="ExternalOutput").ap()

    with tile.TileContext(nc) as tc, ExitStack() as ctx:
        singles = ctx.enter_context(tc.tile_pool(name="singles", bufs=1))
        ld = ctx.enter_context(tc.tile_pool(name="ld", bufs=3))
        vb = ctx.enter_context(tc.tile_pool(name="vb", bufs=3))
        ph = ctx.enter_context(tc.tile_pool(name="ph", bufs=3))
        qt = ctx.enter_context(tc.tile_pool(name="qt", bufs=3))
        rcp = ctx.enter_context(tc.tile_pool(name="rcp", bufs=8))
        ob = ctx.enter_context(tc.tile_pool(name="ob", bufs=3))
        pn = ctx.enter_context(tc.tile_pool(name="pn", bufs=4, space="PSUM"))
        pk = ctx.enter_context(tc.tile_pool(name="pk", bufs=1, space="PSUM"))

        sig_bias = singles.tile([P, 1], f32)
        nc.vector.memset(sig_bias, -4.102)

        # Block-diagonal kv operand per head pair: rows 0:64 cols 0:65 hold
        # kv_ext of the even head, rows 64:128 cols 65:130 the odd head.
        kv_bd = singles.tile([P, G * NPAIR, 2 * W], bf16)
        nc.vector.memset(kv_bd, 0.0)

        kv_ps = [pk.tile([P, GC + 2], f32, tag=f"kv{i}", name=f"kv{i}")
                 for i in range(G * NPAIR)]

        for g in range(G):
            rbase = g * L

            # ---- K/V streaming: kv_pair += phiK_pair^T @ [V|1]_pair ----
            for ib in range(NBS):
                rows = slice(rbase + ib * TBS * P * SUB,
                             rbase + (ib + 1) * TBS * P * SUB)
                k_raw = ld.tile([P, TBS, RT], f32, tag="kraw", name="k_raw")
                nc.sync.dma_start(
                    out=k_raw,
                    in_=K[rows, :].rearrange("(t p s) e -> p t (s e)", p=P, s=SUB),
                )
                phiK = ph.tile([P, TBS, RT], bf16, tag="phiK", name="phiK")
                nc.scalar.activation(
                    out=phiK, in_=k_raw, func=SIG, bias=sig_bias, scale=0.6053
                )
                # [V_group(256) | 1 | 1] lines per (t, s): 512 B contiguous
                # DMA writes (no sub-512B read-modify-write), ones at the
                # tail so one matmul also accumulates k_sum in column 256.
                # Full-tile memset first: supplies the ones and forces the
                # scheduler to order memset -> DMA (overlapping regions).
                v_bf = vb.tile([P, TBS, SUB, GC + 2], bf16, name="v_bf")
                nc.gpsimd.memset(
                    v_bf.rearrange("p t s w -> p (t s) w"), 1.0)
                for t in range(TBS):
                    trows = slice(rbase + (ib * TBS + t) * P * SUB,
                                  rbase + (ib * TBS + t + 1) * P * SUB)
                    nc.gpsimd.dma_start(
                        out=v_bf[:, t, :, 0:GC],
                        in_=V[trows, :].rearrange("(p s) e -> p (s e)", s=SUB),
                    )
                for t in range(TBS):
                    for s in range(SUB):
                        for c in range(NPAIR):
                            nc.tensor.matmul(
                                out=kv_ps[g * NPAIR + c],
                                lhsT=phiK[:, t, s * GC + c * P:
                                          s * GC + (c + 1) * P],
                                rhs=v_bf[:, t, s, :],
                                start=(ib == 0 and t == 0 and s == 0),
                                stop=(ib == NBS - 1 and t == TBS - 1
                                      and s == SUB - 1),
                            )
            for c in range(NPAIR):
                pg = g * NPAIR + c
                nc.vector.tensor_copy(
                    out=kv_bd[0:D, pg, 0:D],
                    in_=kv_ps[pg][0:D, 2 * c * D:(2 * c + 1) * D])
                nc.vector.tensor_copy(
                    out=kv_bd[0:D, pg, D:W],
                    in_=kv_ps[pg][0:D, GC:GC + 1])
                nc.vector.tensor_copy(
                    out=kv_bd[D:P, pg, W:W + D],
                    in_=kv_ps[pg][D:P, (2 * c + 1) * D:(2 * c + 2) * D])
                nc.vector.tensor_copy(
                    out=kv_bd[D:P, pg, W + D:2 * W],
                    in_=kv_ps[pg][D:P, GC:GC + 1])

            # ---- Q phase: QT rows are already phi-transposed layout; one
            # big sigmoid per load, one matmul per 128-q block against the
            # block-diagonal kv, divide on DVE ----
            QB = 2048       # q columns per load batch (1 MiB)
            for c in range(NPAIR):
                erow = g * GC + c * P
                for qb in range(L // QB):
                    qt_raw = ld.tile([P, QB], f32, tag="qtraw", name="qt_raw")
                    nc.sync.dma_start(
                        out=qt_raw,
                        in_=Q[erow:erow + P, qb * QB:(qb + 1) * QB],
                    )
                    qtT = qt.tile([P, QB], bf16, tag="qtT", name="qtT")
                    nc.scalar.activation(
                        out=qtT, in_=qt_raw, func=SIG, bias=sig_bias,
                        scale=0.6053,
                    )
                    out_t = ob.tile([P, QB // P, P], f32, name="out_t")
                    for qk in range(QB // P):
                        num = pn.tile([P, 2, W], f32, tag="num", name="num")
                        nc.tensor.matmul(
                            out=num.rearrange("p a b -> p (a b)"),
                            lhsT=qtT[:, qk * P:(qk + 1) * P],
                            rhs=kv_bd[:, g * NPAIR + c, :],
                        )
                        r = rcp.tile([P, 2], f32, tag="r", name="r")
                        nc.vector.reciprocal(out=r, in_=num[:, :, D])
                        r_bc = bass.AP(
                            tensor=r.tensor, offset=r.offset,
                            ap=[r.ap[0], r.ap[1], [0, D]],
                        )
                        nc.vector.tensor_tensor(
                            out=out_t[:, qk].rearrange("p (a d) -> p a d", a=2),
                            in0=num[:, :, 0:D],
                            in1=r_bc,
                            op=mybir.AluOpType.mult,
                        )
                    obase = (g * NPAIR + c) * L + qb * QB
                    nc.scalar.dma_start(
                        out=O[obase:obase + QB, :].rearrange(
                            "(k p) e -> p k e", p=P),
                        in_=out_t,
                    )

    nc.compile()
    return nc


def _get_nc():
    if "nc" not in _CACHE:
        _CACHE["nc"] = _build_nc()
    return _CACHE["nc"]


def _shard(arr):
    """Full [B, L, E] f32 -> list of 8 per-core [2L, 256] group-stacked."""
    out = []
    for c in range(N_CORES):
        b, g = divmod(c, 2)
        sl = arr[b, :, g * EC:(g + 1) * EC]
        out.append(np.ascontiguousarray(
            np.concatenate([sl[:, 0:GC], sl[:, GC:EC]], axis=0)))
    return out


def _shard_t(arr):
    """Full [B, L, E] f32 -> list of 8 per-core transposed [512, L] slices."""
    out = []
    for c in range(N_CORES):
        b, g = divmod(c, 2)
        out.append(np.ascontiguousarray(arr[b, :, g * EC:(g + 1) * EC].T))
    return out


def _unshard_o(o3):
    """Per-core [4L, 128] (g, c, q-major rows) -> [L, EC] core slice."""
    blocks = o3.reshape(2 * G, L, P)
    return np.concatenate([blocks[i] for i in range(2 * G)], axis=1)


def run_sharded(in_maps, trace=False, trace_cores=None):
    from concourse.bass_utils import run_bass_kernel_spmd

    nc = _get_nc()
    kwargs = {}
    if trace:
        kwargs = dict(trace=True, trace_cores=trace_cores or [0])
    return run_bass_kernel_spmd(nc, in_maps, core_ids=list(range(N_CORES)), **kwargs)


def kernel(**inputs):
    Q = np.asarray(inputs["Q"], dtype=np.float32)
    K = np.asarray(inputs["K"], dtype=np.float32)
    V = np.asarray(inputs["V"], dtype=np.float32)
    qs, ks, vs = _shard_t(Q), _shard(K), _shard(V)
    in_maps = [{"Q": qs[c], "K": ks[c], "V": vs[c]} for c in range(N_CORES)]
    res = run_sharded(in_maps)
    out = np.empty((B, L, E), dtype=np.float32)
    for c in range(N_CORES):
        b, g = divmod(c, 2)
        out[b, :, g * EC:(g + 1) * EC] = _unshard_o(res.results[c]["O"])
    return out



# revision 2
# speedup vs baseline: 1.4834x; 1.4834x over previous
"""Multi-head linear attention on Trainium2 — 8-core SPMD, batch+head sharded.

Full-tensor contract: kernel(**inputs) takes the complete Q/K/V
[4, 4096, 1024] f32 arrays, internally shards them across 8 NeuronCores
(core c -> batch c//2, heads 8*(c%2) .. 8*(c%2)+8, i.e. a contiguous
512-column slice of the embedding dim), runs one Bass kernel per core,
and reassembles the full [4, 4096, 1024] f32 output.

Per-core math (H=8 local heads, D=64, L=4096):
    phi = sigmoid(0.6053*x - 4.102)
    kv_ext[h] = phi_K[h]^T @ [V[h] | 1]     # [64, 65], f32 PSUM accum
    numden[h] = phi_Q[h] @ kv_ext[h]        # [L, 65]
    out[h]    = numden[h][:, :64] / numden[h][:, 64:65]

All device I/O is fp16 (host casts f32 -> fp16 on the way in and fp16 ->
f32 on the way out; matmul accumulation stays f32 in PSUM), halving HBM
traffic to 12.6 MiB of loads + 4 MiB of stores per core.

Host staging per core:
  Q: transposed [512, L] fp16 (partition = embedding dim), so phi_Q^T is
     a straight load + sigmoid.
  K: two 4-head groups stacked along rows -> [2L, 256] fp16; each SBUF
     partition line carries 2 consecutive L-rows (1 KiB descriptors).
  V: head-PAIR-major [4L, 130] fp16 rows [V_pair(128) | 1 | 1] — the
     ones column is baked in on the host, so one matmul per 128-row
     chunk accumulates both kv and k_sum with zero wasted columns
     (rhs is 130 wide instead of the group-wide 258) and no on-device
     memset; 2-row partition lines keep descriptors at 520 B.
  O: [128, 4L] fp16, one 4 KiB-descriptor store per (pair, 2048-q) tile.

Head pairs share matmuls: kv for a pair accumulates in one [128, 130]
PSUM tile (head0 rows 0:64, head1 rows 64:128, k_sum in col 128), and
the Q-phase matmul multiplies against a block-diagonal [128, 130] kv
operand, yielding both heads' num|den per 128-q block.  Division runs
on VectorE batched 3 q-blocks at a time: one strided reciprocal + one
4-D-strided broadcast multiply per PSUM bank.
"""

import numpy as np

B = 4
L = 4096
E = 1024
NH = 8            # heads per core
D = 64
W = D + 1         # head block width incl. ones/den column
EC = NH * D       # 512 embedding columns per core
P = 128
G = 2             # head groups (4 heads each), stacked along rows
GC = EC // G      # 256 columns per group
NPAIR = GC // P   # head pairs per group (2)
SUB = 2           # L-rows per partition line
VW = 2 * W        # 130: pair block width in V staging / kv tiles
RT = SUB * GC     # 512 elements per K partition line
NT = L // (P * SUB)   # 16 tiles (256 L-rows) per group
TBS = 4           # tiles per DMA batch
NBS = NT // TBS   # 4 batches per tensor per group
QB = 2048         # q columns per Q-phase piece
NQB = L // QB     # 2 pieces per pair
N_CORES = 8

_CACHE = {}


def _build_nc():
    from contextlib import ExitStack

    import concourse.bacc as bacc
    import concourse.bass as bass
    import concourse.mybir as mybir
    import concourse.tile as tile

    f32 = mybir.dt.float32
    f16 = mybir.dt.float16
    SIG = mybir.ActivationFunctionType.Sigmoid

    nc = bacc.Bacc("TRN2", target_bir_lowering=False, debug=False)
    Q = nc.dram_tensor("Q", [EC, L], f16, kind="ExternalInput").ap()
    K = nc.dram_tensor("K", [G * L, GC], f16, kind="ExternalInput").ap()
    V = nc.dram_tensor("V", [G * NPAIR * L, VW], f16, kind="ExternalInput").ap()
    O = nc.dram_tensor("O", [P, G * NPAIR * L], f16, kind="ExternalOutput").ap()

    NPG = G * NPAIR   # 4 head pairs total

    with tile.TileContext(nc) as tc, ExitStack() as ctx:
        singles = ctx.enter_context(tc.tile_pool(name="singles", bufs=1))
        ld = ctx.enter_context(tc.tile_pool(name="ld", bufs=3))
        vb = ctx.enter_context(tc.tile_pool(name="vb", bufs=3))
        ph = ctx.enter_context(tc.tile_pool(name="ph", bufs=3))
        qt = ctx.enter_context(tc.tile_pool(name="qt", bufs=3))
        rcp = ctx.enter_context(tc.tile_pool(name="rcp", bufs=6))
        ob = ctx.enter_context(tc.tile_pool(name="ob", bufs=3))
        pn = ctx.enter_context(tc.tile_pool(name="pn", bufs=4, space="PSUM"))
        pk = ctx.enter_context(tc.tile_pool(name="pk", bufs=1, space="PSUM"))

        sig_bias = singles.tile([P, 1], f32)
        nc.vector.memset(sig_bias, -4.102)

        # Block-diagonal kv operand per head pair: rows 0:64 cols 0:65 hold
        # kv_ext of the even head, rows 64:128 cols 65:130 the odd head.
        kv_bd = singles.tile([P, NPG, VW], f16)
        nc.vector.memset(kv_bd, 0.0)

        kv_ps = [pk.tile([P, VW], f32, tag=f"kv{i}", name=f"kv{i}")
                 for i in range(NPG)]

        # Whole-Q resident buffers, one per head pair (8 KiB/partition).
        q_raw = [singles.tile([P, L], f16, tag=f"qr{i}", name=f"qr{i}")
                 for i in range(NPG)]

        def emit_kv_batch(g, ib):
            """Load K batch + per-pair V batches, sigmoid, accumulate kv."""
            rows = slice(g * L + ib * TBS * P * SUB,
                         g * L + (ib + 1) * TBS * P * SUB)
            k_raw = ld.tile([P, TBS, RT], f16, tag="kraw", name="k_raw")
            nc.sync.dma_start(
                out=k_raw,
                in_=K[rows, :].rearrange("(t p s) e -> p t (s e)", p=P, s=SUB),
            )
            # interleave one Q piece behind each K batch on the SP queue
            qi = ib // 2
            if ib % 2 == 0:
                pg = g * NPAIR + qi
                erow = g * GC + qi * P
                nc.sync.dma_start(out=q_raw[pg], in_=Q[erow:erow + P, :])
            phiK = ph.tile([P, TBS, RT], f16, tag="phiK", name="phiK")
            nc.scalar.activation(
                out=phiK, in_=k_raw, func=SIG, bias=sig_bias, scale=0.6053
            )
            v_bf = []
            for c in range(NPAIR):
                pg = g * NPAIR + c
                vrows = slice(pg * L + ib * TBS * P * SUB,
                              pg * L + (ib + 1) * TBS * P * SUB)
                vt = vb.tile([P, TBS, SUB * VW], f16, tag=f"v{c}",
                             name=f"v{c}")
                nc.gpsimd.dma_start(
                    out=vt,
                    in_=V[vrows, :].rearrange("(t p s) e -> p t (s e)",
                                              p=P, s=SUB),
                )
                v_bf.append(vt)
            for t in range(TBS):
                for s in range(SUB):
                    for c in range(NPAIR):
                        nc.tensor.matmul(
                            out=kv_ps[g * NPAIR + c],
                            lhsT=phiK[:, t, s * GC + c * P:s * GC + (c + 1) * P],
                            rhs=v_bf[c][:, t, s * VW:(s + 1) * VW],
                            start=(ib == 0 and t == 0 and s == 0),
                            stop=(ib == NBS - 1 and t == TBS - 1
                                  and s == SUB - 1),
                        )

        def emit_kv_finish(g):
            """Pack the group's kv PSUM tiles into the block-diag operand."""
            for c in range(NPAIR):
                pg = g * NPAIR + c
                nc.vector.tensor_copy(
                    out=kv_bd[0:D, pg, 0:D], in_=kv_ps[pg][0:D, 0:D])
                nc.vector.tensor_copy(
                    out=kv_bd[0:D, pg, D:W], in_=kv_ps[pg][0:D, 2 * D:2 * D + 1])
                nc.vector.tensor_copy(
                    out=kv_bd[D:P, pg, W:W + D], in_=kv_ps[pg][D:P, D:2 * D])
                nc.vector.tensor_copy(
                    out=kv_bd[D:P, pg, W + D:VW], in_=kv_ps[pg][D:P, 2 * D:2 * D + 1])

        def emit_q_piece(g, c, qb):
            """sigmoid 2048 q + 16 matmuls + batched div + one store."""
            pg = g * NPAIR + c
            qtT = qt.tile([P, QB], f16, tag="qtT", name="qtT")
            nc.scalar.activation(
                out=qtT, in_=q_raw[pg][:, qb * QB:(qb + 1) * QB],
                func=SIG, bias=sig_bias, scale=0.6053,
            )
            out_t = ob.tile([P, QB], f16, tag="outt", name="out_t")
            nqk = QB // P               # 16
            qk = 0
            while qk < nqk:
                nb = min(3, nqk - qk)   # triples, remainder 1
                num = pn.tile([P, 3, VW], f32, tag="num", name="num")
                for i in range(nb):
                    nc.tensor.matmul(
                        out=num[:, i, :],
                        lhsT=qtT[:, (qk + i) * P:(qk + i + 1) * P],
                        rhs=kv_bd[:, pg, :],
                    )
                r = rcp.tile([P, 3, 2], f32, tag="r", name="r")
                den = bass.AP(
                    tensor=num.tensor, offset=num.offset + D,
                    ap=[num.ap[0], [VW, nb], [W, 2]],
                )
                nc.vector.reciprocal(out=r[:, 0:nb, :], in_=den)
                nums = bass.AP(
                    tensor=num.tensor, offset=num.offset,
                    ap=[num.ap[0], [VW, nb], [W, 2], [1, D]],
                )
                r_bc = bass.AP(
                    tensor=r.tensor, offset=r.offset,
                    ap=[r.ap[0], [2, nb], [1, 2], [0, D]],
                )
                nc.vector.tensor_tensor(
                    out=out_t[:, qk * P:(qk + nb) * P].rearrange(
                        "p (a b d) -> p a b d", a=nb, b=2),
                    in0=nums, in1=r_bc, op=mybir.AluOpType.mult,
                )
                qk += nb
            obase = pg * L + qb * QB
            nc.scalar.dma_start(out=O[:, obase:obase + QB], in_=out_t)

        # ---- software-pipelined emission ----
        for ib in range(NBS):
            emit_kv_batch(0, ib)
        emit_kv_finish(0)
        # interleave group-1 kv streaming with group-0 Q phase
        pieces0 = [(0, c, qb) for c in range(NPAIR) for qb in range(NQB)]
        for ib in range(NBS):
            emit_kv_batch(1, ib)
            emit_q_piece(*pieces0[ib])
        emit_kv_finish(1)
        for c in range(NPAIR):
            for qb in range(NQB):
                emit_q_piece(1, c, qb)

    nc.compile()
    return nc


def _get_nc():
    if "nc" not in _CACHE:
        _CACHE["nc"] = _build_nc()
    return _CACHE["nc"]


def _shard_q(arr):
    """Full [B, L, E] f32 -> per-core transposed [512, L] fp16 slices."""
    out = []
    for c in range(N_CORES):
        b, g = divmod(c, 2)
        out.append(np.ascontiguousarray(
            arr[b, :, g * EC:(g + 1) * EC].T.astype(np.float16)))
    return out


def _shard_k(arr):
    """Full [B, L, E] f32 -> per-core [2L, 256] fp16 group-stacked."""
    out = []
    for c in range(N_CORES):
        b, g = divmod(c, 2)
        sl = arr[b, :, g * EC:(g + 1) * EC].astype(np.float16)
        out.append(np.ascontiguousarray(
            np.concatenate([sl[:, 0:GC], sl[:, GC:EC]], axis=0)))
    return out


def _shard_v(arr):
    """Full [B, L, E] f32 -> per-core pair-major [4L, 130] fp16 with
    ones baked into columns 128:130."""
    out = []
    for c in range(N_CORES):
        b, g = divmod(c, 2)
        sl = arr[b, :, g * EC:(g + 1) * EC].astype(np.float16)
        st = np.ones((G * NPAIR * L, VW), dtype=np.float16)
        for pg in range(G * NPAIR):
            st[pg * L:(pg + 1) * L, 0:P] = sl[:, pg * P:(pg + 1) * P]
        out.append(st)
    return out


def _unshard_o(o):
    """Per-core [128, 4L] fp16 -> [L, EC] f32 core slice."""
    blocks = o.reshape(P, G * NPAIR, NQB, QB // P, P)   # [p, pg, qb, qk, e]
    # q = qb*QB + qk*P + p
    perm = blocks.transpose(1, 2, 3, 0, 4).reshape(G * NPAIR, L, P)
    return np.concatenate(list(perm), axis=1).astype(np.float32)


def run_sharded(in_maps, trace=False, trace_cores=None):
    from concourse.bass_utils import run_bass_kernel_spmd

    nc = _get_nc()
    kwargs = {}
    if trace:
        kwargs = dict(trace=True, trace_cores=trace_cores or [0])
    return run_bass_kernel_spmd(nc, in_maps, core_ids=list(range(N_CORES)), **kwargs)


def kernel(**inputs):
    Q = np.asarray(inputs["Q"], dtype=np.float32)
    K = np.asarray(inputs["K"], dtype=np.float32)
    V = np.asarray(inputs["V"], dtype=np.float32)
    qs, ks, vs = _shard_q(Q), _shard_k(K), _shard_v(V)
    in_maps = [{"Q": qs[c], "K": ks[c], "V": vs[c]} for c in range(N_CORES)]
    res = run_sharded(in_maps)
    out = np.empty((B, L, E), dtype=np.float32)
    for c in range(N_CORES):
        b, g = divmod(c, 2)
        out[b, :, g * EC:(g + 1) * EC] = _unshard_o(res.results[c]["O"])
    return out
